# revision 13
# baseline (speedup 1.0000x reference)
"""Trainium2 Bass kernel for nn_MultiHeadDensityRatioEstimator (v2).

Math: logits l_h(i,j) = -log1p(sq_h(i,j)); w_h = 1/v_h with v = 1+sq;
savg = sum_h l_h = ln(prod_h w_h). All logsumexps become plain sums of w.

v2 layout (vs the transposed v1): pair tiles are [128 zy-rows i, 2048 zx
cols j] per head, so the per-(i,h) rowsums ride the free axis:
  - host pre-packs augmented f32r matmul operands (zero device preproc)
  - PE: one K=18 matmul per (head, j-512-chunk) -> PSUM v tile [128,2048]
  - reciprocal+rowsum in one pass: ScalarE ACT Reciprocal with accum_out
    (6 heads/group) + custom 7-stage DVE RECIP_SUM_ANT (2 heads/group)
  - savg: 7-mul bf16 product tree on DVE (231 G elem/s when GpSimd idle),
    software-pipelined one group behind the recips
  - GpSimd does nothing in the main loop (it poisons the shared SBUF port)
  - tail: tiny AllReduce of per-head sums overlapped with the Ln sweeps,
    then sigmoid/count sweeps; 16 partial stats out; host combines.
"""

import math
import sys

import numpy as np

for _p in ("/opt/trn_rl_repo",):
    if _p not in sys.path:
        sys.path.insert(0, _p)

N = 4096
D = 128
H = 8
DH = 16
NCORES = 8
RPC = N // NCORES  # 512 zy rows per core
NIB = RPC // 128  # 4 i-chunks
NJH = 2  # j halves of 2048
FDH = 2048  # head-tile free dim
LOG_NN1 = float(np.log(float(N) * (N - 1)))
NSTAT = 32
NSC = 20  # stats tile cols

# packed operand slots: head h -> tensor HT[h], slot HS[h]
HT = [0, 0, 0, 1, 1, 1, 2, 2]
HS = [0, 1, 2, 0, 1, 2, 0, 1]

# heads whose recip+rowsum runs on DVE (rest on ScalarE)
DVE_HEADS = (6, 7)

# 7-stage quadratic-minimax reciprocal constants (see register_recip_sum)
RSC = dict(s0=-0.706651166, s1=-0.166336546, imm2=-0.0130421322)
RECIP_SUM_SHAS = {"v3": "3c868abbaecb0fa9", "v4": "01e39383903d81a1"}


def register_recip_sum():
    """RECIP_SUM_ANT: out = recip7(in0), accum_out = sum(out) along free.

    recip7: 1/x ~= (~x)*(a + p*(b + p*c)) with p = x*bitcast(~x) in
    [-4.5, -4]; 7 ALU stages leave stage 8 free for the accumulator
    (the stock 2-NR RECIPROCAL_APPROX_FAST needs all 8). Max rel err 8.4e-5.
    """
    from operator import add
    import concourse.dve_ops as dve_ops
    from concourse.dve_spec import C0, C1, C2, Bin, AluOp, Spec, Src0
    from concourse.dve_ops import DveOp

    for op in dve_ops.OPS:
        if op.name == "RECIP_SUM_ANT":
            return op

    _n = Bin(AluOp.BITWISE_NOT, Src0, Src0)
    _p = Src0 * _n

    def _ref(in0, in1, c0, c1, c2):
        nx = (~in0.view(np.int32)).view(np.float32)
        p = (in0 * nx).astype(np.float32)
        b = (nx * (c0 + p * (c1 + p * c2))).astype(np.float32)
        return b, b.reshape(b.shape[0], -1).sum(axis=-1, keepdims=True)

    op = DveOp(
        "RECIP_SUM_ANT",
        Spec(body=_n * (C0 + _p * (C1 + _p * C2)), accum=add, reference=_ref),
        subdim=False,
        uops_sha=dict(RECIP_SUM_SHAS),
    )
    dve_ops.OPS.append(op)
    dve_ops.CUSTOM_DVE_SPECS[op.name] = op.spec
    dve_ops._SUB_OPCODE_FOR_NAME[op.name] = (
        dve_ops._CUSTOM_DVE_ROW_BASE + len(dve_ops.OPS) - 1
    )
    return op


def act_raw(nc, out, in_, func, bias=0.0, scale=1.0, accum_out=None):
    """Raw InstActivation emit (bypasses the Reciprocal accuracy guard;
    measured max rel err 1.2e-5 on our v>=1 inputs)."""
    from concourse import mybir

    se = nc.scalar
    inputs = [se.lower_ap(in_)]
    for arg in (bias, scale, 0.0):
        inputs.append(mybir.ImmediateValue(dtype=mybir.dt.float32, value=arg))
    outputs = [se.lower_ap(out)]
    if accum_out is not None:
        outputs.append(se.lower_ap(accum_out))
    return se.add_instruction(
        mybir.InstActivation(
            name=se.bass.get_next_instruction_name(),
            func=func,
            ins=inputs,
            outs=outputs,
        )
    )


def build_bass():
    import concourse.bacc as bacc
    import concourse.tile as tile
    from concourse import mybir

    RS = register_recip_sum()

    f32 = mybir.dt.float32
    f32r = mybir.dt.float32r
    bf16 = mybir.dt.bfloat16
    AF = mybir.ActivationFunctionType
    ALU = mybir.AluOpType
    AX = mybir.AxisListType

    nc = bacc.Bacc("TRN2", num_devices=NCORES, debug=False)

    # host-packed operands (bf16: halves DMA bytes; PE cadence identical)
    xb = nc.dram_tensor("xb", [96, 3 * N], bf16, kind="ExternalInput")
    yb = nc.dram_tensor("yb", [96, 3 * RPC], bf16, kind="ExternalInput")
    wdd = nc.dram_tensor("wd", [128, NIB * H], f32, kind="ExternalInput")
    out = nc.dram_tensor("out", [1, NSTAT], f32, kind="ExternalOutput")

    from contextlib import ExitStack

    with tile.TileContext(nc) as tc, ExitStack() as stk:
        per = stk.enter_context(tc.tile_pool(name="per", bufs=1))

        XB = per.tile([96, 3 * N], bf16, name="XB")
        YB = per.tile([96, 3 * RPC], bf16, name="YB")
        WD = per.tile([128, NIB * H], f32, name="WD")
        Qst = [per.tile([128, N], bf16, name=f"Qst{i}") for i in range(NIB)]
        rs = per.tile([128, 64], f32, name="rs")
        stats = per.tile([128, NSC], f32, name="stats")
        ones128 = per.tile([128, 1], f32)
        ones1 = per.tile([1, 128], f32)

        nc.vector.memset(stats[:], 0.0)
        nc.vector.memset(ones128[:], 1.0)
        nc.vector.memset(ones1[:], 1.0)

        # input DMAs ordered by first use, spread over 5 queues
        def xchunk(q, t, jh):
            q.dma_start(
                out=XB[:, t * N + jh * FDH : t * N + (jh + 1) * FDH],
                in_=xb[:, t * N + jh * FDH : t * N + (jh + 1) * FDH],
            )

        for t in range(3):
            nc.sync.dma_start(
                out=YB[:, t * RPC : (t + 1) * RPC],
                in_=yb[:, t * RPC : (t + 1) * RPC],
            )
        xchunk(nc.scalar, 0, 0)
        xchunk(nc.sync, 0, 1)
        xchunk(nc.scalar, 1, 0)
        xchunk(nc.sync, 1, 1)
        xchunk(nc.gpsimd, 2, 0)
        xchunk(nc.gpsimd, 2, 1)
        nc.gpsimd.dma_start(out=WD[:], in_=wdd[:])

        # warm up the collective machinery during the main loop
        with tc.tile_pool(name="warm", bufs=1, space="DRAM") as wdp:
            wsb = per.tile([1, 1], f32, name="wsb")
            nc.vector.memset(wsb[:], 0.0)
            win = wdp.tile([1, 1], f32, tag="win")
            wout_ = wdp.tile([1, 1], f32, tag="wout")
            nc.sync.dma_start(out=win[:], in_=wsb[:])
            nc.gpsimd.collective_compute(
                "AllReduce",
                mybir.AluOpType.add,
                replica_groups=[list(range(NCORES))],
                ins=[win.opt()],
                outs=[wout_.opt()],
            )

        # ---------------- main loop ----------------
        # ic-major, h, jh-inner: the 8 matmuls of one (ic, h) share lhsT so
        # only the first pays the unshadowed LDWEIGHTS. Tree muls fire as
        # their w pairs complete (per jh), keeping the W pool at 2 gens.
        with (
            tc.tile_pool(name="vp", bufs=2, space="PSUM") as vp,
            tc.tile_pool(name="wp", bufs=2) as wp,
            tc.tile_pool(name="up", bufs=2) as up,
            tc.tile_pool(name="qp", bufs=2) as qp,
        ):
            # order B: per-tile Sc:DVE alternation 3:1; heads grouped so the
            # tree pairs close in completion order: (0,1),(2,6) then (3,4),(5,7)
            BLOCKS = [
                ([0, 1, 2, 6], 0), ([0, 1, 2, 6], 1),
                ([3, 4, 5, 7], 0), ([3, 4, 5, 7], 1),
            ]
            for ic in range(NIB):
                W = {}
                U = {}
                Q = {}
                for bi, (heads, jh) in enumerate(BLOCKS):
                    g = jh * NIB + ic
                    for h in heads:
                        t, s = HT[h], HS[h]
                        ps = vp.tile([128, FDH], f32, tag="v", name="ps")
                        for q in range(4):
                            nc.tensor.matmul(
                                out=ps[:, q * 512 : (q + 1) * 512],
                                lhsT=YB[
                                    32 * s : 32 * s + 18,
                                    t * RPC + ic * 128 : t * RPC + (ic + 1) * 128,
                                ],
                                rhs=XB[
                                    32 * s : 32 * s + 18,
                                    t * N + jh * FDH + q * 512 : t * N
                                    + jh * FDH
                                    + (q + 1) * 512,
                                ],
                            )
                        w = wp.tile(
                            [128, FDH], bf16, tag=f"w{h}", name=f"W{h}"
                        )
                        W[(h, jh)] = w
                        col = rs[:, g * 8 + h : g * 8 + h + 1]
                        if h in DVE_HEADS:
                            nc.vector._custom_dve(
                                RS, out=w[:], in0=ps[:],
                                s0=RSC["s0"], s1=RSC["s1"], imm2=RSC["imm2"],
                                accum_out=col,
                            )
                        else:
                            act_raw(
                                nc, w[:], ps[:], AF.Reciprocal, accum_out=col
                            )
                        # eager tree on DVE, completion-order pairs
                        if h == 1:
                            u = up.tile([128, FDH], bf16, tag="u0", name="u0")
                            nc.vector.tensor_mul(
                                u[:], W[(0, jh)][:], W[(1, jh)][:]
                            )
                            U[(0, jh)] = u
                        elif h == 6:
                            u = up.tile([128, FDH], bf16, tag="u1", name="u1")
                            nc.vector.tensor_mul(
                                u[:], W[(2, jh)][:], W[(6, jh)][:]
                            )
                            U[(1, jh)] = u
                            qa = qp.tile([128, FDH], bf16, tag="qa", name="qa")
                            nc.vector.tensor_mul(
                                qa[:], U[(0, jh)][:], U[(1, jh)][:]
                            )
                            Q[(0, jh)] = qa
                        elif h == 4:
                            u = up.tile([128, FDH], bf16, tag="u2", name="u2")
                            nc.vector.tensor_mul(
                                u[:], W[(3, jh)][:], W[(4, jh)][:]
                            )
                            U[(2, jh)] = u
                        elif h == 7:
                            u = up.tile([128, FDH], bf16, tag="u3", name="u3")
                            nc.vector.tensor_mul(
                                u[:], W[(5, jh)][:], W[(7, jh)][:]
                            )
                            U[(3, jh)] = u
                            qb = qp.tile([128, FDH], bf16, tag="qb", name="qb")
                            nc.vector.tensor_mul(
                                qb[:], U[(2, jh)][:], U[(3, jh)][:]
                            )
                            nc.vector.tensor_mul(
                                Qst[ic][:, jh * FDH : (jh + 1) * FDH],
                                Q[(0, jh)][:], qb[:],
                            )

        # ---------------- tail ----------------
        # Pre-collective: local baseline bl_local from this core's S partials,
        # Ln sweeps, then sigmoid sums G0 and sum-of-sigma^2 G1 against
        # bl_local (host Taylor-corrects to the true baseline). The tiny
        # AllReduce runs concurrently. Post-collective only the counts run,
        # via sigma > theta with theta = 0.5 + db/4 (db = bl_true - bl_local
        # from a log1p polynomial on the S ratios - no ACT table switches).
        from concourse.dve_ops import RECIP_APPROX_FAST_CONSTS, RECIPROCAL_APPROX_FAST

        RCF = RECIP_APPROX_FAST_CONSTS
        with (
            tc.tile_pool(name="fp", bufs=1, space="PSUM") as fp,
            tc.tile_pool(name="fs", bufs=1) as fs,
            tc.tile_pool(name="fs2", bufs=2) as fs2,
            tc.tile_pool(name="dram", bufs=1, space="DRAM") as dp,
        ):
            # off-diagonal per-(i,h) rowsums: jh0 + jh1 - w_diag
            RS32 = fs.tile([128, 32], f32)
            nc.vector.tensor_add(RS32[:], rs[:, 0:32], rs[:, 32:64])
            nc.vector.tensor_sub(RS32[:], RS32[:], WD[:])
            R8 = fs.tile([128, 8], f32)
            nc.vector.tensor_reduce(
                out=R8[:], in_=RS32.rearrange("p (a h) -> p h a", h=8),
                axis=AX.X, op=ALU.add,
            )
            S1 = fp.tile([1, 8], f32, tag="s1")
            nc.tensor.matmul(out=S1[:], lhsT=ones128[:, 0:1], rhs=R8[:])
            Scc = fs.tile([1, 8], f32)
            nc.vector.tensor_copy(Scc[:], S1[:])
            cc_in = dp.tile([1, 8], f32, tag="ccin")
            cc_out = dp.tile([1, 8], f32, tag="ccout")
            nc.sync.dma_start(out=cc_in[:], in_=Scc[:])
            nc.gpsimd.collective_compute(
                "AllReduce",
                mybir.AluOpType.add,
                replica_groups=[list(range(NCORES))],
                ins=[cc_in.opt()],
                outs=[cc_out.opt()],
            )
            Sg = fs.tile([1, 8], f32)
            nc.sync.dma_start(out=Sg[:], in_=cc_out[:])

            # ln sweeps (bf16 out -> ScalarE 2x) + rep term + local baseline
            LT = [fs.tile([128, N], f32, name=f"LT{i}") for i in range(NIB)]
            LR32 = fs.tile([128, 32], f32)
            for ic in range(NIB):
                nc.scalar.activation(
                    out=LT[ic][:], in_=Qst[ic][:], func=AF.Ln,
                    accum_out=stats[:, ic : ic + 1],
                )
            nc.scalar.activation(
                out=LR32[:], in_=RS32[:], func=AF.Ln, accum_out=stats[:, 12:13]
            )
            SL8 = fs.tile([1, 8], f32)
            nc.scalar.activation(out=SL8[:], in_=Scc[:], func=AF.Ln)
            SLs = fs.tile([1, 1], f32)
            nc.vector.tensor_reduce(out=SLs[:], in_=SL8[:], axis=AX.X, op=ALU.add)
            bll = fs.tile([1, 1], f32)
            lnn1 = fs.tile([1, 1], f32)
            nc.vector.memset(lnn1[:], float(np.log(8.0)) - LOG_NN1)
            nc.scalar.activation(
                out=bll[:], in_=SLs[:], func=AF.Identity, scale=1.0 / H,
                bias=lnn1[:],
            )
            psB = fp.tile([128, 1], f32, tag="psB")
            nc.tensor.matmul(out=psB[:], lhsT=ones1[0:1, :], rhs=bll[0:1, :])
            nbl = fs.tile([128, 1], f32)
            nc.scalar.activation(out=nbl[:], in_=psB[:], func=AF.Copy, scale=-1.0)

            # sigmoid sums G0 (bf16 sigma tiles) + G1 = sum sigma^2 on DVE
            SGT = [fs.tile([128, N], bf16, name=f"SGT{i}") for i in range(NIB)]
            for ic in range(NIB):
                nc.scalar.activation(
                    out=SGT[ic][:], in_=LT[ic][:], func=AF.Sigmoid,
                    scale=1.0 / H, bias=nbl[:],
                    accum_out=stats[:, 4 + ic : 5 + ic],
                )
                s2 = fs2.tile([128, N], bf16, tag="s2")
                nc.scalar.activation(
                    out=s2[:], in_=SGT[ic][:], func=AF.Square,
                    accum_out=stats[:, 14 + ic : 15 + ic],
                )

            # post-collective (DVE only - ScalarE keeps streaming sigmoids):
            # dbt = sum_h ln(Sg_h/(8*Scc_h)) via log1p polynomial, then the
            # exact count threshold 8*bl_true = 8*bl_local + dbt
            rS = fs.tile([1, 8], f32)
            nc.vector._custom_dve(
                RECIPROCAL_APPROX_FAST, out=rS[:], in0=Scc[:],
                s0=RCF["s0"], s1=RCF["s1"], imm2=RCF["imm2"],
            )
            ratio = fs.tile([1, 8], f32)
            nc.vector.tensor_mul(ratio[:], Sg[:], rS[:])
            uu = fs.tile([1, 8], f32)
            nc.vector.tensor_scalar(
                out=uu[:], in0=ratio[:], scalar1=0.125, scalar2=-1.0,
                op0=ALU.mult, op1=ALU.add,
            )
            u2 = fs.tile([1, 8], f32)
            nc.vector.tensor_mul(u2[:], uu[:], uu[:])
            u3 = fs.tile([1, 8], f32)
            nc.vector.tensor_mul(u3[:], u2[:], uu[:])
            t2t = fs.tile([1, 8], f32)
            nc.vector.tensor_scalar(
                out=t2t[:], in0=u2[:], scalar1=-0.5, scalar2=None, op0=ALU.mult
            )
            t3t = fs.tile([1, 8], f32)
            nc.vector.tensor_scalar(
                out=t3t[:], in0=u3[:], scalar1=1.0 / 3.0, scalar2=None,
                op0=ALU.mult,
            )
            lsum = fs.tile([1, 8], f32)
            nc.vector.tensor_add(lsum[:], uu[:], t2t[:])
            nc.vector.tensor_add(lsum[:], lsum[:], t3t[:])
            dbt = fs.tile([1, 1], f32)
            nc.vector.tensor_reduce(out=dbt[:], in_=lsum[:], axis=AX.X, op=ALU.add)
            t8t = fs.tile([1, 1], f32)
            nc.vector.tensor_scalar(
                out=t8t[:], in0=bll[:], scalar1=8.0, scalar2=None, op0=ALU.mult
            )
            nc.vector.tensor_add(t8t[:], t8t[:], dbt[:])
            psT = fp.tile([128, 1], f32, tag="psT")
            nc.tensor.matmul(out=psT[:], lhsT=ones1[0:1, :], rhs=t8t[0:1, :])
            thb = fs.tile([128, 1], f32)
            nc.vector.tensor_copy(thb[:], psT[:])
            for ic in range(NIB):
                cn = fs2.tile([128, N], bf16, tag="cn")
                nc.vector.tensor_scalar(
                    out=cn[:], in0=LT[ic][:], scalar1=thb[:, 0:1], scalar2=None,
                    op0=ALU.is_gt, op1=ALU.add,
                    accum_out=stats[:, 8 + ic : 9 + ic],
                )

            psO = fp.tile([1, NSC], f32, tag="psO")
            nc.tensor.matmul(out=psO[:], lhsT=ones128[:, 0:1], rhs=stats[:])
            outrow = fs.tile([1, NSTAT], f32)
            nc.vector.memset(outrow[:], 0.0)
            nc.scalar.activation(out=outrow[:, 0:NSC], in_=psO[:], func=AF.Copy)
            nc.scalar.activation(out=outrow[:, 20:28], in_=Scc[:], func=AF.Copy)
            nc.scalar.activation(
                out=outrow[:, 28:29], in_=dbt[:, 0:1], func=AF.Copy,
                scale=1.0 / 8.0,
            )
            nc.sync.dma_start(out=out[:], in_=outrow[:])

    nc.compile()
    return nc


_CACHED_NC = None


def _get_nc():
    global _CACHED_NC
    if _CACHED_NC is None:
        _CACHED_NC = build_bass()
    return _CACHED_NC


def _pack_host(z_x, z_y):
    """Host-side operand packing. Returns (xb [96,3N] f32, per-core list of
    (yb [96,3*RPC] f32, wd [128,32] f32))."""
    zx = np.ascontiguousarray(z_x, dtype=np.float32)
    zy = np.ascontiguousarray(z_y, dtype=np.float32)

    xb = np.zeros((96, 3 * N), np.float32)
    for h in range(H):
        t, s = HT[h], HS[h]
        blk = zx[:, h * DH : (h + 1) * DH]  # [N, 16]
        xb[32 * s : 32 * s + 16, t * N : (t + 1) * N] = -2.0 * blk.T
        xb[32 * s + 16, t * N : (t + 1) * N] = 1.0
        xb[32 * s + 17, t * N : (t + 1) * N] = (
            (blk.astype(np.float64) ** 2).sum(1) + 0.5
        ).astype(np.float32)

    cores = []
    for c in range(NCORES):
        zyc = zy[c * RPC : (c + 1) * RPC]  # [512, 128]
        ybc = np.zeros((96, 3 * RPC), np.float32)
        for h in range(H):
            t, s = HT[h], HS[h]
            blk = zyc[:, h * DH : (h + 1) * DH]
            ybc[32 * s : 32 * s + 16, t * RPC : (t + 1) * RPC] = blk.T
            ybc[32 * s + 16, t * RPC : (t + 1) * RPC] = (
                (blk.astype(np.float64) ** 2).sum(1) + 0.5
            ).astype(np.float32)
            ybc[32 * s + 17, t * RPC : (t + 1) * RPC] = 1.0
        dz = (zyc - zx[c * RPC : (c + 1) * RPC]).astype(np.float64)
        wd = np.zeros((128, NIB * H), np.float32)
        for ic in range(NIB):
            for h in range(H):
                d2 = (dz[ic * 128 : (ic + 1) * 128, h * DH : (h + 1) * DH] ** 2).sum(1)
                wd[:, ic * H + h] = (1.0 / (1.0 + d2)).astype(np.float32)
        cores.append((ybc, wd))
    return xb, cores


def make_in_maps(z_x, z_y):
    import ml_dtypes

    xb, cores = _pack_host(z_x, z_y)
    xb16 = np.ascontiguousarray(xb.astype(ml_dtypes.bfloat16))
    return [
        {"xb": xb16, "yb": np.ascontiguousarray(ybc.astype(ml_dtypes.bfloat16)),
         "wd": wd}
        for (ybc, wd) in cores
    ]


def combine(stats, z_x, z_y):
    """stats: [NCORES, NSTAT]; returns the 9 reference outputs.

    Device cols: 0-3 slq, 4-7 G0 (sigma sums vs bl_local), 8-11 counts
    (theta-corrected, exact vs bl_true), 12 rep, 14-17 G1 (sum sigma^2),
    20-27 per-core S_h partials, 28 device db (debug).
    """
    st = stats.astype(np.float64)
    slq = st[:, 0:4].sum()
    rep_sum = st[:, 12].sum()
    cnt_full = st[:, 8:12].sum()

    S_core = st[:, 20:28]  # [NCORES, H]
    S_glob = S_core.sum(0)
    blavg = float(np.log(S_glob).mean() - LOG_NN1)

    # Taylor-correct the sigma sums from each core's local baseline
    sig_full = 0.0
    for c in range(NCORES):
        bl_local = float(np.log(8.0 * S_core[c]).mean() - LOG_NN1)
        db = blavg - bl_local
        G0 = st[c, 4:8].sum()
        G1 = G0 - st[c, 14:18].sum()  # sum sigma' = sum sigma - sum sigma^2
        sig_full += G0 - db * G1

    zx = z_x.astype(np.float64)
    zy = z_y.astype(np.float64)
    dz = zy - zx
    ld = np.zeros(N, np.float64)
    for h in range(H):
        d2 = (dz[:, h * DH : (h + 1) * DH] ** 2).sum(1)
        ld -= np.log1p(d2)
    sum_ld = ld.sum()
    sig_diag = (1.0 / (1.0 + np.exp(-(ld / H - blavg)))).sum()
    cp = float((ld > H * blavg).sum())

    mean_pos = sum_ld / (H * N) - blavg
    mean_neg = (slq - sum_ld) / (H * N * (N - 1)) - blavg
    mean_sig_pos = sig_diag / N
    mean_sig_neg = (sig_full - sig_diag) / (N * (N - 1))
    cn = cnt_full - cp
    acc = (cp + (N * (N - 1) - cn)) / (N * N)
    recall = cp / N
    tpfp = cp + cn
    precision = (cp / max(tpfp, 1.0)) if tpfp > 0 else 0.0
    rep_mean = rep_sum / (H * N) - math.log(N - 1) - blavg
    decay = 0.01 * (np.mean(zx * zx) + np.mean(zy * zy))
    loss = -mean_pos + rep_mean + decay
    return np.array(
        [
            mean_pos, mean_neg, mean_sig_pos, mean_sig_neg, acc, recall,
            precision, blavg, loss,
        ],
        dtype=np.float32,
    )


def run_on_hw(z_x, z_y, trace=False):
    from concourse.bass_utils import run_bass_kernel_spmd

    nc = _get_nc()
    res = run_bass_kernel_spmd(
        nc, make_in_maps(z_x, z_y), core_ids=list(range(NCORES)), trace=trace
    )
    stats = np.stack([r["out"][0] for r in res.results])
    return combine(stats, z_x, z_y), res


def kernel(z_x, z_y):
    out, _ = run_on_hw(z_x, z_y, trace=False)
    return out
